# revision 55
# baseline (speedup 1.0000x reference)
"""Causal self-attention Bass kernel for 8x Trainium2 NeuronCores.

Problem: B=8, T=1024, D=1024, H=16 heads (head_dim 64), fp32.
Sharding: data parallel over batch -- each of the 8 cores handles one
batch element with replicated weights; outputs are stacked on the host.

v2 design notes (vs the 340us baseline):
  * The baseline's PE stream had ~300-850ns micro-gaps in the attention
    phase (waiting on ACT exp), which re-throttled the PE HAM clock gate
    to 1.2 GHz for ~52% of the kernel.  Here the qkv/proj "producer"
    matmuls are interleaved into the attention instruction stream so the
    PE always has independent ready work and stays at 2.4 GHz.
  * Head pairs share the QK step: q/k for heads (2h, 2h+1) live at
    partitions 0-63 / 64-127 of the same qk f-tile, so their K=64 QK
    matmuls go to distinct PE row groups (tile_position auto-derived)
    and execute concurrently -- QK cost per pair ~halves.
  * The two tq-block (j) loops are interleaved per head pair so the
    producer stream spans the whole attention phase; the j=0 projection
    pumps into the last pair's j=1 attention.
  * Softmax 1/denominator uses the DVE fast-reciprocal custom op (from
    an SBUF staging copy -- reading PSUM directly from the custom op, a
    2-bank PSUM score tile, and gpsimd elementwise ops all misbehave on
    real HW even though CoreSim accepts them).
  * All bias applications ride on DVE drains (tensor_scalar / broadcast
    tiles) instead of K=1 ones-row matmuls (-16k PE rows).
  * Inputs stream over three DMA queues (sync/gpsimd/scalar) in
    first-use order: x0-3 half-chunks race in for the transposes, wq
    before x4-7, wv next, w_proj last; bias vectors load as [1,D] rows
    and are partition-broadcast on-device.
  * Measured on HW: 279.4 us (baseline 340.7 us), rel err 0.0034.
"""

import numpy as np
from collections import deque
from contextlib import ExitStack

import concourse.bass as bass
import concourse.bacc as bacc
import concourse.tile as tile
import concourse.mybir as mybir
from concourse import bass_utils

F32 = mybir.dt.float32
BF16 = mybir.dt.bfloat16
AF = mybir.ActivationFunctionType
OP = mybir.AluOpType

B, T, D, H, HD = 8, 1024, 1024, 16, 64
P = 128
N_CORES = 8

TRACE = False
_CACHE = {}
LAST_RESULT = {}

# Producer pump rates (PE filler matmuls consumed per attention i-step).
PUMP_J0 = 2
PUMP_J1 = 2
LDW_OPT = False  # walrus rejects ldw-opt for some of our LDWEIGHTS shapes


def _patch_ldw_opt():
    """walrus is invoked with --enable-ldw-opt=false; flipping it lets
    codegen elide LDWEIGHTS for consecutive matmuls sharing a stationary
    operand (loops are ordered to maximize that)."""
    if not LDW_OPT or getattr(bass_utils, "_ldw_patched", False):
        return
    orig = bass_utils.run_command

    def run_command_ldw(argv, **kw):
        argv = ["--enable-ldw-opt=true" if a == "--enable-ldw-opt=false" else a
                for a in argv]
        return orig(argv, **kw)

    bass_utils.run_command = run_command_ldw
    bass_utils._ldw_patched = True


def _build_tile_kernel(nc, aps):
    x, wq, bq, wp, ident, tri, bqv_bc, bp_bc, out = (
        aps["x"], aps["w_qkv"], aps["b_qkv"], aps["w_proj"],
        aps["ident"], aps["tri"], aps["bqv_bc"], aps["bp_bc"], aps["out"],
    )

    with tile.TileContext(nc) as tc, ExitStack() as ctx:
        consts = ctx.enter_context(tc.tile_pool(name="consts", bufs=1))
        wq_pool = ctx.enter_context(tc.tile_pool(name="wq_pool", bufs=8))
        wp_pool = ctx.enter_context(tc.tile_pool(name="wp_pool", bufs=8))
        xn_pool = ctx.enter_context(tc.tile_pool(name="xn_pool", bufs=4))
        xt_pool = ctx.enter_context(tc.tile_pool(name="xt_pool", bufs=16))
        qk_pool = ctx.enter_context(tc.tile_pool(name="qk_pool", bufs=16))
        v_pool = ctx.enter_context(tc.tile_pool(name="v_pool", bufs=8))
        p_pool = ctx.enter_context(tc.tile_pool(name="p_pool", bufs=3))
        at_pool = ctx.enter_context(tc.tile_pool(name="at_pool", bufs=16))
        nrm_pool = ctx.enter_context(tc.tile_pool(name="nrm_pool", bufs=4))
        rb_pool = ctx.enter_context(tc.tile_pool(name="rb_pool", bufs=4))
        y_pool = ctx.enter_context(tc.tile_pool(name="y_pool", bufs=3))
        ps_s = ctx.enter_context(tc.tile_pool(name="ps_s", bufs=3, space="PSUM"))
        ps_o = ctx.enter_context(tc.tile_pool(name="ps_o", bufs=3, space="PSUM"))
        ps_pr = ctx.enter_context(tc.tile_pool(name="ps_pr", bufs=2, space="PSUM"))

        # ---- input DMAs, spread across engine issue queues ---------------
        id_sb = consts.tile([P, P], F32)
        nc.scalar.dma_start(out=id_sb, in_=ident)
        # x races in on two queues, then the q/k weight halves (needed
        # first), then the v weight parts, then w_proj (needed last).
        xns = [xn_pool.tile([P, D], F32, name="xn", tag="xn") for c in range(8)]
        wqc = [wq_pool.tile([P, 2 * D], BF16, name="wt", tag="wq")
               for k in range(8)]
        wvc = [wq_pool.tile([P, D], BF16, name="wv", tag="wv")
               for k in range(8)]
        wpc = [wp_pool.tile([P, D], BF16, name="wpt", tag="wp")
               for c in range(8)]

        def ld_x(eng, c):
            eng.dma_start(out=xns[c], in_=x[c * P:(c + 1) * P, :])

        def ld_wq(eng, k):
            eng.dma_start(out=wqc[k], in_=wq[k * P:(k + 1) * P, 0:2048])

        def ld_wv(eng, k):
            eng.dma_start(out=wvc[k], in_=wq[k * P:(k + 1) * P, 2048:3072])

        def ld_wp(eng, c):
            eng.dma_start(out=wpc[c], in_=wp[c * P:(c + 1) * P, :])

        # 3-queue schedule ordered by when each tensor is first consumed:
        # sync+gpsimd race x0-3 (half-chunks) then stream wq; the scalar
        # queue's head is dedicated to x4-7 (for the jj=1 transposes), then
        # wv; w_proj (needed last) fills the tails.
        for c in range(4):
            nc.sync.dma_start(out=xns[c][:, 0:512],
                              in_=x[c * P:(c + 1) * P, 0:512])
            nc.gpsimd.dma_start(out=xns[c][:, 512:1024],
                                in_=x[c * P:(c + 1) * P, 512:1024])
        for k in range(0, 8, 2):
            ld_wq(nc.sync, k)
            ld_wq(nc.gpsimd, k + 1)
        for k in range(0, 8, 2):
            ld_wv(nc.sync, k)
            ld_wv(nc.gpsimd, k + 1)
        for c in range(0, 8, 2):
            ld_wp(nc.sync, c)
            ld_wp(nc.gpsimd, c + 1)
        for c in range(4, 8):
            ld_x(nc.scalar, c)
        tri_sb = consts.tile([P, P], BF16)
        nc.scalar.dma_start(out=tri_sb, in_=tri)
        bcol_sb = consts.tile([P, 16], F32)  # b_qkv[0:2048] as per-partition cols
        nc.scalar.dma_start(out=bcol_sb, in_=bq[0:2048].rearrange("(f p) -> p f", p=P))
        # bias rows come in as [1, D] and are partition-broadcast on-device
        bqv_r = consts.tile([1, D], F32)
        nc.scalar.dma_start(out=bqv_r, in_=bqv_bc)
        bp_r = consts.tile([1, D], F32)
        nc.scalar.dma_start(out=bp_r, in_=bp_bc)
        bqv_sb = consts.tile([P, D], F32)  # v bias broadcast across partitions
        nc.gpsimd.partition_broadcast(bqv_sb, bqv_r)
        bp_sb = consts.tile([P, D], F32)  # proj bias broadcast across partitions
        nc.gpsimd.partition_broadcast(bp_sb, bp_r)

        # ---- phase 1a: x -> xT (PE transpose of 128x128 blocks) ----------
        xt_tiles = {}  # (k, jj) -> [128, 512] bf16 = xT[k*128:.., jj*512:..]

        def transposes(jj):
            for k in range(8):
                pst = ps_pr.tile([P, 512], F32, name="pst", tag="pr")
                for tt in range(4):
                    nc.tensor.transpose(
                        pst[:, tt * P:(tt + 1) * P],
                        xns[jj * 4 + tt][:, k * P:(k + 1) * P],
                        id_sb,
                    )
                xt_t = xt_pool.tile([P, 512], BF16, name="xt_t", tag="xt")
                nc.vector.tensor_copy(xt_t, pst)
                xt_tiles[(k, jj)] = xt_t

        # ---- producers: qkT f-tiles and v tiles, emitted as thunks -------
        qk_tiles = {}
        for f in list(range(8)) + list(range(8, 16)):
            qk_tiles[f] = qk_pool.tile([P, T], BF16, name="qk_t", tag="qk")

        def qkT_half_units(f, half):
            """9 thunks producing half of qk_tiles[f]."""
            st = {}
            units = []

            def mk_mm(k):
                def go():
                    if k == 0:
                        st[0] = ps_pr.tile([P, 512], F32, name="qa", tag="pr")
                    nc.tensor.matmul(
                        st[0],
                        wqc[k][:, f * P:(f + 1) * P],
                        xt_tiles[(k, half)],
                        start=(k == 0), stop=(k == 7),
                    )
                return go
            units.extend(mk_mm(k) for k in range(8))

            def drain():
                nc.vector.tensor_scalar_add(
                    qk_tiles[f][:, half * 512:(half + 1) * 512],
                    st.pop(0), bcol_sb[:, f:f + 1],
                )
            units.append(drain)
            return units

        def qkT_units(f):
            """18 thunks producing qk_tiles[f]; both tq-halves accumulate in
            lockstep so consecutive matmuls share the stationary operand
            (ldw-opt elides the second LDWEIGHTS)."""
            st = {}
            units = []
            for k in range(8):
                def mk_mm(k=k):
                    def go():
                        if k == 0:
                            st[0] = ps_pr.tile([P, 512], F32, name="qa", tag="pr")
                            st[1] = ps_pr.tile([P, 512], F32, name="qb", tag="pr")
                        wsl = wqc[k][:, f * P:(f + 1) * P]
                        nc.tensor.matmul(st[0], wsl, xt_tiles[(k, 0)],
                                         start=(k == 0), stop=(k == 7))
                        nc.tensor.matmul(st[1], wsl, xt_tiles[(k, 1)],
                                         start=(k == 0), stop=(k == 7))
                    return go
                units.append(mk_mm())

            def drain(half):
                nc.vector.tensor_scalar_add(
                    qk_tiles[f][:, half * 512:(half + 1) * 512],
                    st.pop(half), bcol_sb[:, f:f + 1],
                )
            units.append(lambda: drain(0))
            units.append(lambda: drain(1))
            return units

        v_tiles = []
        for m in range(8):
            vt = v_pool.tile([P, 16 * 65], BF16, name="vt", tag="v")
            v_tiles.append(vt)

        def v_units(m):
            """ones-col memset + 8 dual matmuls + biased drains for v_tiles[m];
            the two w-halves accumulate in lockstep to share the stationary."""
            st = {}
            units = []

            def ones():
                nc.vector.memset(
                    v_tiles[m].rearrange("p (h c) -> p h c", c=65)[:, :, 64:65],
                    1.0,
                )
            units.append(ones)
            for k in range(8):
                def mk_mm(k=k):
                    def go():
                        if k == 0:
                            st[0] = ps_pr.tile([P, 512], F32, name="va", tag="pr")
                            st[1] = ps_pr.tile([P, 512], F32, name="vb", tag="pr")
                        xsl = xt_tiles[(k, m // 4)][:, (m % 4) * P:(m % 4 + 1) * P]
                        nc.tensor.matmul(st[0], xsl, wvc[k][:, 0:512],
                                         start=(k == 0), stop=(k == 7))
                        nc.tensor.matmul(st[1], xsl, wvc[k][:, 512:1024],
                                         start=(k == 0), stop=(k == 7))
                    return go
                units.append(mk_mm())

            def drain(n):
                rr = v_tiles[m].rearrange("p (h c) -> p h c", c=65)
                nc.vector.tensor_tensor(
                    rr[:, n * 8:(n + 1) * 8, 0:64],
                    st.pop(n), bqv_sb[:, n * 512:(n + 1) * 512],
                    op=OP.add,
                )
            units.append(lambda: drain(0))
            units.append(lambda: drain(1))
            return units

        att_tiles = {}
        for j in range(2):
            for hp in range(8):
                att_tiles[(hp, j)] = at_pool.tile(
                    [P, 512], BF16, name="at", tag="at")

        def proj_units(j):
            """projection of tq block j: per mi, 16 matmuls + biased drains."""
            units = []
            for mi in range(4):
                st = {}
                for c in range(8):
                    def mk_mm(c=c, mi=mi, st=st):
                        def go():
                            if c == 0:
                                st[0] = ps_pr.tile(
                                    [P, 512], F32, name="y0", tag="pr")
                                st[1] = ps_o.tile(
                                    [P, 512], F32, name="y1", tag="o")
                            asl = att_tiles[(c, j)][:, mi * P:(mi + 1) * P]
                            nc.tensor.matmul(st[0], asl, wpc[c][:, 0:512],
                                             start=(c == 0), stop=(c == 7))
                            nc.tensor.matmul(st[1], asl, wpc[c][:, 512:1024],
                                             start=(c == 0), stop=(c == 7))
                        return go
                    units.append(mk_mm())

                def drain(mi=mi, st=st):
                    mrow = 4 * j + mi
                    for n in range(2):
                        y_sb = y_pool.tile([P, 512], F32, name="y_sb", tag="y")
                        nc.vector.tensor_tensor(
                            y_sb, st[0] if n == 0 else st[1],
                            bp_sb[:, n * 512:(n + 1) * 512], op=OP.add,
                        )
                        nc.sync.dma_start(
                            out=out[mrow * P:(mrow + 1) * P,
                                    n * 512:(n + 1) * 512],
                            in_=y_sb,
                        )
                    st.clear()
                units.append(drain)
            return units

        # Producer queue. qkT f-pairs (0,8) and (1,9) plus all v tiles are
        # emitted inline up front (attention for hp=0/1 needs them); the
        # rest are pumped into the attention stream.
        prod = deque()
        qk_done = [False] * 16

        def emit_qk(f):
            for u in qkT_units(f):
                u()
            qk_done[f] = True

        # startup: transposes trickle behind the 2-queue x load; the f=0/8
        # half-0 matmuls run while x4-7 + wv stream in; v trickles last.
        transposes(0)
        for f in (0, 8):
            for u in qkT_half_units(f, 0):
                u()
        transposes(1)
        for f in (0, 8):
            for u in qkT_half_units(f, 1):
                u()
        qk_done[0] = qk_done[8] = True
        for m in range(8):
            for u in v_units(m):
                u()
        emit_qk(1), emit_qk(9)
        for f in range(2, 8):
            prod.append(("qk", f))
            prod.append(("qk", f + 8))

        pending_units = deque()

        def pump(n):
            for _ in range(n):
                if pending_units:
                    pending_units.popleft()()
                    continue
                if not prod:
                    return
                kind, arg = prod.popleft()
                if kind == "qk":
                    pending_units.extend(qkT_units(arg))
                    qk_done[arg] = "emitting"
                    pending_units.append(lambda f=arg: qk_done.__setitem__(f, True))
                else:
                    pending_units.extend(proj_units(arg))
                if pending_units:
                    pending_units.popleft()()

        def ensure_qk(f):
            # flush until qk_tiles[f] fully emitted
            while qk_done[f] is not True:
                if pending_units:
                    pending_units.popleft()()
                elif prod:
                    pump(1)
                else:
                    raise RuntimeError(f"qk {f} never produced")

        # ---- attention ---------------------------------------------------
        def attention(j, hp, rate):
            h0 = 2 * hp
            qT = qk_tiles[hp]
            kT = qk_tiles[8 + hp]
            ni = 4 * j + 4
            o0 = ps_o.tile([P, 512], F32, name="o0", tag="o")
            o1 = ps_o.tile([P, 512], F32, name="o1", tag="o")
            pend = None
            for i in range(ni):
                m = i - 4 * j
                ws = min(P * m, 256) if m >= 0 else 0
                s0 = ps_s.tile([P, 512], F32, name="s0", tag="s")
                s1 = ps_s.tile([P, 512], F32, name="s1", tag="s")
                nc.tensor.matmul(
                    s0[:, ws:], kT[0:64, i * P:(i + 1) * P],
                    qT[0:64, j * 512 + ws:(j + 1) * 512],
                    start=True, stop=True,
                )
                nc.tensor.matmul(
                    s1[:, ws:], kT[64:128, i * P:(i + 1) * P],
                    qT[64:128, j * 512 + ws:(j + 1) * 512],
                    start=True, stop=True,
                )
                pump(rate)
                p = p_pool.tile([P, 1024], BF16, name="p", tag="p")
                nc.scalar.activation(p[:, ws:512], s0[:, ws:], AF.Exp, scale=0.125)
                nc.scalar.activation(p[:, 512 + ws:], s1[:, ws:], AF.Exp, scale=0.125)
                if m >= 0:
                    if m == 3:
                        nc.vector.memset(p[:, 256:384], 0.0)
                        nc.vector.memset(p[:, 768:896], 0.0)
                    dc = P * m
                    nc.vector.tensor_tensor(
                        p[:, dc:dc + P], p[:, dc:dc + P], tri_sb, op=OP.mult)
                    nc.vector.tensor_tensor(
                        p[:, 512 + dc:512 + dc + P], p[:, 512 + dc:512 + dc + P],
                        tri_sb, op=OP.mult)
                if pend is not None:
                    pi, pws, pp = pend
                    rr = v_tiles[pi].rearrange("p (h c) -> p h c", c=65)
                    nc.tensor.matmul(o0[0:65, pws:], rr[:, h0, :],
                                     pp[:, pws:512], start=(pi == 0), stop=False)
                    nc.tensor.matmul(o1[0:65, pws:], rr[:, h0 + 1, :],
                                     pp[:, 512 + pws:], start=(pi == 0), stop=False)
                pend = (i, ws, p)
            pi, pws, pp = pend
            rr = v_tiles[pi].rearrange("p (h c) -> p h c", c=65)
            nc.tensor.matmul(o0[0:65, pws:], rr[:, h0, :],
                             pp[:, pws:512], start=(pi == 0), stop=True)
            nc.tensor.matmul(o1[0:65, pws:], rr[:, h0 + 1, :],
                             pp[:, 512 + pws:], start=(pi == 0), stop=True)
            # normalization, decoupled from the PSUM banks: o and the
            # denominators drain to SBUF on DVE (fast bank release), 1/d via
            # the DVE fast-reciprocal custom op (no ACT involvement), then
            # broadcast + multiply run entirely on the idle gpsimd engine.
            at = att_tiles[(hp, j)]
            dd0 = nrm_pool.tile([1, 512], F32, name="dd0", tag="nrm")
            dd1 = nrm_pool.tile([1, 512], F32, name="dd1", tag="nrm")
            nc.vector.tensor_copy(dd0, o0[64:65, :])
            nc.vector.tensor_copy(dd1, o1[64:65, :])
            r0 = nrm_pool.tile([1, 512], F32, name="r0", tag="nrm")
            r1 = nrm_pool.tile([1, 512], F32, name="r1", tag="nrm")
            nc.vector.reciprocal_approx_fast(r0, dd0)
            nc.vector.reciprocal_approx_fast(r1, dd1)
            rb0 = rb_pool.tile([64, 512], F32, name="rb0", tag="rb")
            rb1 = rb_pool.tile([64, 512], F32, name="rb1", tag="rb")
            nc.gpsimd.partition_broadcast(rb0, r0)
            nc.gpsimd.partition_broadcast(rb1, r1)
            nc.vector.tensor_tensor(at[0:64, :], o0[0:64, :], rb0, op=OP.mult)
            nc.vector.tensor_tensor(at[64:128, :], o1[0:64, :], rb1, op=OP.mult)

        # j-blocks interleaved per head pair so the qkT producer stream
        # spreads over the whole attention phase (one f-pair per hp); the
        # j=0 projection pumps into the last pair's j=1 attention.
        for hp in range(8):
            ensure_qk(hp), ensure_qk(8 + hp)
            attention(0, hp, PUMP_J0)
            if hp == 7:
                prod.append(("proj", 0))
            attention(1, hp, PUMP_J1)
        pump(10 ** 6)  # flush leftovers
        for u in proj_units(1):
            u()


def _pin_act_table(arch):
    """Force every ACT func we use into one table so walrus never emits
    mid-kernel ACT_TABLE_LOADs (each is ~1.3us on the ScalarE stream)."""
    import concourse.hw_specs as hw_specs
    tabs = hw_specs.get_activation_tables(arch)
    keep = "natural_log_exp_and_others"
    if keep not in tabs:
        return
    need = tabs[keep] & {AF.Exp, AF.Ln, AF.Copy, AF.Identity}
    for name, fns in tabs.items():
        if name != keep:
            fns -= need


def _get_nc():
    if "nc" in _CACHE:
        return _CACHE["nc"]
    nc = bacc.Bacc("TRN2", target_bir_lowering=False, debug=False,
                   num_devices=N_CORES)
    _pin_act_table(nc.m.arch)
    _patch_ldw_opt()
    aps = {
        "x": nc.dram_tensor("x", [T, D], F32, kind="ExternalInput").ap(),
        "w_qkv": nc.dram_tensor("w_qkv", [D, 3 * D], BF16, kind="ExternalInput").ap(),
        "b_qkv": nc.dram_tensor("b_qkv", [3 * D], F32, kind="ExternalInput").ap(),
        "w_proj": nc.dram_tensor("w_proj", [D, D], BF16, kind="ExternalInput").ap(),
        "ident": nc.dram_tensor("ident", [P, P], F32, kind="ExternalInput").ap(),
        "tri": nc.dram_tensor("tri", [P, P], BF16, kind="ExternalInput").ap(),
        "bqv_bc": nc.dram_tensor("bqv_bc", [1, D], F32, kind="ExternalInput").ap(),
        "bp_bc": nc.dram_tensor("bp_bc", [1, D], F32, kind="ExternalInput").ap(),
        "out": nc.dram_tensor("out", [T, D], F32, kind="ExternalOutput").ap(),
    }
    _build_tile_kernel(nc, aps)
    nc.compile()
    _CACHE["nc"] = nc
    return nc


def _host_consts():
    import ml_dtypes
    ident = np.eye(P, dtype=np.float32)
    r = np.arange(P)
    tri = (r[:, None] <= r[None, :]).astype(ml_dtypes.bfloat16)
    return ident, tri


def kernel(x, w_qkv, b_qkv, w_proj, b_proj):
    x = np.ascontiguousarray(np.asarray(x, dtype=np.float32))
    w_qkv = np.ascontiguousarray(np.asarray(w_qkv, dtype=np.float32))
    b_qkv = np.ascontiguousarray(np.asarray(b_qkv, dtype=np.float32))
    w_proj = np.ascontiguousarray(np.asarray(w_proj, dtype=np.float32))
    b_proj = np.ascontiguousarray(np.asarray(b_proj, dtype=np.float32))

    nc = _get_nc()
    import ml_dtypes
    bf = ml_dtypes.bfloat16
    ident, tri = _host_consts()
    wq_bf = w_qkv.astype(bf)
    wp_bf = w_proj.astype(bf)
    bqv_bc = np.ascontiguousarray(b_qkv[2048:3072].reshape(1, D))
    bp_bc = np.ascontiguousarray(b_proj.reshape(1, D))
    in_maps = [
        {
            "x": x[b],
            "w_qkv": wq_bf,
            "b_qkv": b_qkv,
            "w_proj": wp_bf,
            "ident": ident,
            "tri": tri,
            "bqv_bc": bqv_bc,
            "bp_bc": bp_bc,
        }
        for b in range(N_CORES)
    ]
    res = bass_utils.run_bass_kernel_spmd(
        nc, in_maps, core_ids=list(range(N_CORES)), trace=TRACE
    )
    LAST_RESULT["res"] = res
    return np.stack([res.results[c]["out"] for c in range(N_CORES)]).astype(
        np.float32
    )


# revision 58
# speedup vs baseline: 1.0097x; 1.0097x over previous
"""Causal self-attention Bass kernel for 8x Trainium2 NeuronCores.

Problem: B=8, T=1024, D=1024, H=16 heads (head_dim 64), fp32.
Sharding: data parallel over batch -- each of the 8 cores handles one
batch element with replicated weights; outputs are stacked on the host.

v2 design notes (vs the 340us baseline):
  * The baseline's PE stream had ~300-850ns micro-gaps in the attention
    phase (waiting on ACT exp), which re-throttled the PE HAM clock gate
    to 1.2 GHz for ~52% of the kernel.  Here the qkv/proj "producer"
    matmuls are interleaved into the attention instruction stream so the
    PE always has independent ready work and stays at 2.4 GHz.
  * Head pairs share the QK step: q/k for heads (2h, 2h+1) live at
    partitions 0-63 / 64-127 of the same qk f-tile, so their K=64 QK
    matmuls go to distinct PE row groups (tile_position auto-derived)
    and execute concurrently -- QK cost per pair ~halves.
  * The two tq-block (j) loops are interleaved per head pair so the
    producer stream spans the whole attention phase; the j=0 projection
    pumps into the last pair's j=1 attention.
  * Softmax 1/denominator uses the DVE fast-reciprocal custom op (from
    an SBUF staging copy -- reading PSUM directly from the custom op, a
    2-bank PSUM score tile, and gpsimd elementwise ops all misbehave on
    real HW even though CoreSim accepts them).
  * All bias applications ride on DVE drains (tensor_scalar / broadcast
    tiles) instead of K=1 ones-row matmuls (-16k PE rows).
  * Inputs stream over three DMA queues (sync/gpsimd/scalar) in
    first-use order: x0-3 half-chunks race in for the transposes, wq
    before x4-7, wv next, w_proj last; bias vectors load as [1,D] rows
    and are partition-broadcast on-device.
  * Measured on HW: 279.4 us (baseline 340.7 us), rel err 0.0034.
"""

import numpy as np
from collections import deque
from contextlib import ExitStack

import concourse.bass as bass
import concourse.bacc as bacc
import concourse.tile as tile
import concourse.mybir as mybir
from concourse import bass_utils

F32 = mybir.dt.float32
BF16 = mybir.dt.bfloat16
AF = mybir.ActivationFunctionType
OP = mybir.AluOpType

B, T, D, H, HD = 8, 1024, 1024, 16, 64
P = 128
N_CORES = 8

TRACE = False
_CACHE = {}
LAST_RESULT = {}

# Producer pump rates (PE filler matmuls consumed per attention i-step).
PUMP_J0 = 2
PUMP_J1 = 2
LDW_OPT = False  # walrus rejects ldw-opt for some of our LDWEIGHTS shapes


def _patch_ldw_opt():
    """walrus is invoked with --enable-ldw-opt=false; flipping it lets
    codegen elide LDWEIGHTS for consecutive matmuls sharing a stationary
    operand (loops are ordered to maximize that)."""
    if not LDW_OPT or getattr(bass_utils, "_ldw_patched", False):
        return
    orig = bass_utils.run_command

    def run_command_ldw(argv, **kw):
        argv = ["--enable-ldw-opt=true" if a == "--enable-ldw-opt=false" else a
                for a in argv]
        return orig(argv, **kw)

    bass_utils.run_command = run_command_ldw
    bass_utils._ldw_patched = True


def _build_tile_kernel(nc, aps):
    x, wq, bq, wp, ident, tri, bqv_bc, bp_bc, out = (
        aps["x"], aps["w_qkv"], aps["b_qkv"], aps["w_proj"],
        aps["ident"], aps["tri"], aps["bqv_bc"], aps["bp_bc"], aps["out"],
    )

    with tile.TileContext(nc) as tc, ExitStack() as ctx:
        consts = ctx.enter_context(tc.tile_pool(name="consts", bufs=1))
        wq_pool = ctx.enter_context(tc.tile_pool(name="wq_pool", bufs=8))
        wp_pool = ctx.enter_context(tc.tile_pool(name="wp_pool", bufs=8))
        xn_pool = ctx.enter_context(tc.tile_pool(name="xn_pool", bufs=4))
        xt_pool = ctx.enter_context(tc.tile_pool(name="xt_pool", bufs=16))
        qk_pool = ctx.enter_context(tc.tile_pool(name="qk_pool", bufs=16))
        v_pool = ctx.enter_context(tc.tile_pool(name="v_pool", bufs=8))
        p_pool = ctx.enter_context(tc.tile_pool(name="p_pool", bufs=3))
        at_pool = ctx.enter_context(tc.tile_pool(name="at_pool", bufs=16))
        nrm_pool = ctx.enter_context(tc.tile_pool(name="nrm_pool", bufs=4))
        rb_pool = ctx.enter_context(tc.tile_pool(name="rb_pool", bufs=4))
        y_pool = ctx.enter_context(tc.tile_pool(name="y_pool", bufs=3))
        ps_s = ctx.enter_context(tc.tile_pool(name="ps_s", bufs=3, space="PSUM"))
        ps_o = ctx.enter_context(tc.tile_pool(name="ps_o", bufs=3, space="PSUM"))
        ps_pr = ctx.enter_context(tc.tile_pool(name="ps_pr", bufs=2, space="PSUM"))

        # ---- input DMAs, spread across engine issue queues ---------------
        id_sb = consts.tile([P, P], F32)
        nc.scalar.dma_start(out=id_sb, in_=ident)
        # x races in on two queues, then the q/k weight halves (needed
        # first), then the v weight parts, then w_proj (needed last).
        xns = [xn_pool.tile([P, D], F32, name="xn", tag="xn") for c in range(8)]
        wqc = [wq_pool.tile([P, 2 * D], BF16, name="wt", tag="wq")
               for k in range(8)]
        wvc = [wq_pool.tile([P, D], BF16, name="wv", tag="wv")
               for k in range(8)]
        wpc = [wp_pool.tile([P, D], BF16, name="wpt", tag="wp")
               for c in range(8)]

        def ld_x(eng, c):
            eng.dma_start(out=xns[c], in_=x[c * P:(c + 1) * P, :])

        def ld_wq_head(eng, k):
            # f-tiles 0,1 (q of heads 0-3) and 8,9 (their k): the columns
            # the first two attention pairs need, raced in first.
            eng.dma_start(out=wqc[k][:, 0:256],
                          in_=wq[k * P:(k + 1) * P, 0:256])
            eng.dma_start(out=wqc[k][:, 1024:1280],
                          in_=wq[k * P:(k + 1) * P, 1024:1280])

        def ld_wq_rest(eng, k):
            eng.dma_start(out=wqc[k][:, 256:1024],
                          in_=wq[k * P:(k + 1) * P, 256:1024])
            eng.dma_start(out=wqc[k][:, 1280:2048],
                          in_=wq[k * P:(k + 1) * P, 1280:2048])

        def ld_wv(eng, k):
            eng.dma_start(out=wvc[k], in_=wq[k * P:(k + 1) * P, 2048:3072])

        def ld_wp(eng, c):
            eng.dma_start(out=wpc[c], in_=wp[c * P:(c + 1) * P, :])

        # 3-queue schedule ordered by when each tensor is first consumed:
        # sync+gpsimd race x0-3 (half-chunks) then stream wq; the scalar
        # queue's head is dedicated to x4-7 (for the jj=1 transposes), then
        # wv; w_proj (needed last) fills the tails.
        for c in range(4):
            nc.sync.dma_start(out=xns[c][:, 0:512],
                              in_=x[c * P:(c + 1) * P, 0:512])
            nc.gpsimd.dma_start(out=xns[c][:, 512:1024],
                                in_=x[c * P:(c + 1) * P, 512:1024])
        for k in range(0, 8, 2):
            ld_wq_head(nc.sync, k)
            ld_wq_head(nc.gpsimd, k + 1)
        for k in range(0, 8, 2):
            ld_wv(nc.sync, k)
            ld_wv(nc.gpsimd, k + 1)
        for k in range(0, 8, 2):
            ld_wq_rest(nc.sync, k)
            ld_wq_rest(nc.gpsimd, k + 1)
        for c in range(0, 8, 2):
            ld_wp(nc.sync, c)
            ld_wp(nc.gpsimd, c + 1)
        for c in range(4, 8):
            ld_x(nc.scalar, c)
        tri_sb = consts.tile([P, P], BF16)
        nc.scalar.dma_start(out=tri_sb, in_=tri)
        bcol_sb = consts.tile([P, 16], F32)  # b_qkv[0:2048] as per-partition cols
        nc.scalar.dma_start(out=bcol_sb, in_=bq[0:2048].rearrange("(f p) -> p f", p=P))
        # bias rows come in as [1, D] and are partition-broadcast on-device
        bqv_r = consts.tile([1, D], F32)
        nc.scalar.dma_start(out=bqv_r, in_=bqv_bc)
        bp_r = consts.tile([1, D], F32)
        nc.scalar.dma_start(out=bp_r, in_=bp_bc)
        bqv_sb = consts.tile([P, D], F32)  # v bias broadcast across partitions
        nc.gpsimd.partition_broadcast(bqv_sb, bqv_r)
        bp_sb = consts.tile([P, D], F32)  # proj bias broadcast across partitions
        nc.gpsimd.partition_broadcast(bp_sb, bp_r)

        # ---- phase 1a: x -> xT (PE transpose of 128x128 blocks) ----------
        xt_tiles = {}  # (k, jj) -> [128, 512] bf16 = xT[k*128:.., jj*512:..]

        def transposes(jj):
            for k in range(8):
                pst = ps_pr.tile([P, 512], F32, name="pst", tag="pr")
                for tt in range(4):
                    nc.tensor.transpose(
                        pst[:, tt * P:(tt + 1) * P],
                        xns[jj * 4 + tt][:, k * P:(k + 1) * P],
                        id_sb,
                    )
                xt_t = xt_pool.tile([P, 512], BF16, name="xt_t", tag="xt")
                nc.vector.tensor_copy(xt_t, pst)
                xt_tiles[(k, jj)] = xt_t

        # ---- producers: qkT f-tiles and v tiles, emitted as thunks -------
        qk_tiles = {}
        for f in list(range(8)) + list(range(8, 16)):
            qk_tiles[f] = qk_pool.tile([P, T], BF16, name="qk_t", tag="qk")

        def qkT_half_units(f, half):
            """9 thunks producing half of qk_tiles[f]."""
            st = {}
            units = []

            def mk_mm(k):
                def go():
                    if k == 0:
                        st[0] = ps_pr.tile([P, 512], F32, name="qa", tag="pr")
                    nc.tensor.matmul(
                        st[0],
                        wqc[k][:, f * P:(f + 1) * P],
                        xt_tiles[(k, half)],
                        start=(k == 0), stop=(k == 7),
                    )
                return go
            units.extend(mk_mm(k) for k in range(8))

            def drain():
                nc.vector.tensor_scalar_add(
                    qk_tiles[f][:, half * 512:(half + 1) * 512],
                    st.pop(0), bcol_sb[:, f:f + 1],
                )
            units.append(drain)
            return units

        def qkT_units(f):
            """18 thunks producing qk_tiles[f]; both tq-halves accumulate in
            lockstep so consecutive matmuls share the stationary operand
            (ldw-opt elides the second LDWEIGHTS)."""
            st = {}
            units = []
            for k in range(8):
                def mk_mm(k=k):
                    def go():
                        if k == 0:
                            st[0] = ps_pr.tile([P, 512], F32, name="qa", tag="pr")
                            st[1] = ps_pr.tile([P, 512], F32, name="qb", tag="pr")
                        wsl = wqc[k][:, f * P:(f + 1) * P]
                        nc.tensor.matmul(st[0], wsl, xt_tiles[(k, 0)],
                                         start=(k == 0), stop=(k == 7))
                        nc.tensor.matmul(st[1], wsl, xt_tiles[(k, 1)],
                                         start=(k == 0), stop=(k == 7))
                    return go
                units.append(mk_mm())

            def drain(half):
                nc.vector.tensor_scalar_add(
                    qk_tiles[f][:, half * 512:(half + 1) * 512],
                    st.pop(half), bcol_sb[:, f:f + 1],
                )
            units.append(lambda: drain(0))
            units.append(lambda: drain(1))
            return units

        v_tiles = []
        for m in range(8):
            vt = v_pool.tile([P, 16 * 65], BF16, name="vt", tag="v")
            v_tiles.append(vt)

        def v_units(m):
            """ones-col memset + 8 dual matmuls + biased drains for v_tiles[m];
            the two w-halves accumulate in lockstep to share the stationary."""
            st = {}
            units = []

            def ones():
                nc.vector.memset(
                    v_tiles[m].rearrange("p (h c) -> p h c", c=65)[:, :, 64:65],
                    1.0,
                )
            units.append(ones)
            for k in range(8):
                def mk_mm(k=k):
                    def go():
                        if k == 0:
                            st[0] = ps_pr.tile([P, 512], F32, name="va", tag="pr")
                            st[1] = ps_pr.tile([P, 512], F32, name="vb", tag="pr")
                        xsl = xt_tiles[(k, m // 4)][:, (m % 4) * P:(m % 4 + 1) * P]
                        nc.tensor.matmul(st[0], xsl, wvc[k][:, 0:512],
                                         start=(k == 0), stop=(k == 7))
                        nc.tensor.matmul(st[1], xsl, wvc[k][:, 512:1024],
                                         start=(k == 0), stop=(k == 7))
                    return go
                units.append(mk_mm())

            def drain(n):
                rr = v_tiles[m].rearrange("p (h c) -> p h c", c=65)
                nc.vector.tensor_tensor(
                    rr[:, n * 8:(n + 1) * 8, 0:64],
                    st.pop(n), bqv_sb[:, n * 512:(n + 1) * 512],
                    op=OP.add,
                )
            units.append(lambda: drain(0))
            units.append(lambda: drain(1))
            return units

        att_tiles = {}
        for j in range(2):
            for hp in range(8):
                att_tiles[(hp, j)] = at_pool.tile(
                    [P, 512], BF16, name="at", tag="at")

        def proj_units(j):
            """projection of tq block j: per mi, 16 matmuls + biased drains."""
            units = []
            for mi in range(4):
                st = {}
                for c in range(8):
                    def mk_mm(c=c, mi=mi, st=st):
                        def go():
                            if c == 0:
                                st[0] = ps_pr.tile(
                                    [P, 512], F32, name="y0", tag="pr")
                                st[1] = ps_o.tile(
                                    [P, 512], F32, name="y1", tag="o")
                            asl = att_tiles[(c, j)][:, mi * P:(mi + 1) * P]
                            nc.tensor.matmul(st[0], asl, wpc[c][:, 0:512],
                                             start=(c == 0), stop=(c == 7))
                            nc.tensor.matmul(st[1], asl, wpc[c][:, 512:1024],
                                             start=(c == 0), stop=(c == 7))
                        return go
                    units.append(mk_mm())

                def drain(mi=mi, st=st):
                    mrow = 4 * j + mi
                    for n in range(2):
                        y_sb = y_pool.tile([P, 512], F32, name="y_sb", tag="y")
                        nc.vector.tensor_tensor(
                            y_sb, st[0] if n == 0 else st[1],
                            bp_sb[:, n * 512:(n + 1) * 512], op=OP.add,
                        )
                        nc.sync.dma_start(
                            out=out[mrow * P:(mrow + 1) * P,
                                    n * 512:(n + 1) * 512],
                            in_=y_sb,
                        )
                    st.clear()
                units.append(drain)
            return units

        # Producer queue. qkT f-pairs (0,8) and (1,9) plus all v tiles are
        # emitted inline up front (attention for hp=0/1 needs them); the
        # rest are pumped into the attention stream.
        prod = deque()
        qk_done = [False] * 16

        def emit_qk(f):
            for u in qkT_units(f):
                u()
            qk_done[f] = True

        # startup: transposes trickle behind the 2-queue x load; the f=0/8
        # half-0 matmuls run while x4-7 + wv stream in; v trickles last.
        transposes(0)
        for f in (0, 8):
            for u in qkT_half_units(f, 0):
                u()
        transposes(1)
        for f in (0, 8):
            for u in qkT_half_units(f, 1):
                u()
        qk_done[0] = qk_done[8] = True
        emit_qk(1), emit_qk(9)
        for m in range(8):
            for u in v_units(m):
                u()
        for f in range(2, 8):
            prod.append(("qk", f))
            prod.append(("qk", f + 8))

        pending_units = deque()

        def pump(n):
            for _ in range(n):
                if pending_units:
                    pending_units.popleft()()
                    continue
                if not prod:
                    return
                kind, arg = prod.popleft()
                if kind == "qk":
                    pending_units.extend(qkT_units(arg))
                    qk_done[arg] = "emitting"
                    pending_units.append(lambda f=arg: qk_done.__setitem__(f, True))
                else:
                    pending_units.extend(proj_units(arg))
                if pending_units:
                    pending_units.popleft()()

        def ensure_qk(f):
            # flush until qk_tiles[f] fully emitted
            while qk_done[f] is not True:
                if pending_units:
                    pending_units.popleft()()
                elif prod:
                    pump(1)
                else:
                    raise RuntimeError(f"qk {f} never produced")

        # ---- attention ---------------------------------------------------
        def attention(j, hp, rate):
            h0 = 2 * hp
            qT = qk_tiles[hp]
            kT = qk_tiles[8 + hp]
            ni = 4 * j + 4
            o0 = ps_o.tile([P, 512], F32, name="o0", tag="o")
            o1 = ps_o.tile([P, 512], F32, name="o1", tag="o")
            pend = None
            for i in range(ni):
                m = i - 4 * j
                ws = min(P * m, 256) if m >= 0 else 0
                s0 = ps_s.tile([P, 512], F32, name="s0", tag="s")
                s1 = ps_s.tile([P, 512], F32, name="s1", tag="s")
                nc.tensor.matmul(
                    s0[:, ws:], kT[0:64, i * P:(i + 1) * P],
                    qT[0:64, j * 512 + ws:(j + 1) * 512],
                    start=True, stop=True,
                )
                nc.tensor.matmul(
                    s1[:, ws:], kT[64:128, i * P:(i + 1) * P],
                    qT[64:128, j * 512 + ws:(j + 1) * 512],
                    start=True, stop=True,
                )
                pump(rate)
                p = p_pool.tile([P, 1024], BF16, name="p", tag="p")
                nc.scalar.activation(p[:, ws:512], s0[:, ws:], AF.Exp, scale=0.125)
                nc.scalar.activation(p[:, 512 + ws:], s1[:, ws:], AF.Exp, scale=0.125)
                if m >= 0:
                    if m == 3:
                        nc.vector.memset(p[:, 256:384], 0.0)
                        nc.vector.memset(p[:, 768:896], 0.0)
                    dc = P * m
                    nc.vector.tensor_tensor(
                        p[:, dc:dc + P], p[:, dc:dc + P], tri_sb, op=OP.mult)
                    nc.vector.tensor_tensor(
                        p[:, 512 + dc:512 + dc + P], p[:, 512 + dc:512 + dc + P],
                        tri_sb, op=OP.mult)
                if pend is not None:
                    pi, pws, pp = pend
                    rr = v_tiles[pi].rearrange("p (h c) -> p h c", c=65)
                    nc.tensor.matmul(o0[0:65, pws:], rr[:, h0, :],
                                     pp[:, pws:512], start=(pi == 0), stop=False)
                    nc.tensor.matmul(o1[0:65, pws:], rr[:, h0 + 1, :],
                                     pp[:, 512 + pws:], start=(pi == 0), stop=False)
                pend = (i, ws, p)
            pi, pws, pp = pend
            rr = v_tiles[pi].rearrange("p (h c) -> p h c", c=65)
            nc.tensor.matmul(o0[0:65, pws:], rr[:, h0, :],
                             pp[:, pws:512], start=(pi == 0), stop=True)
            nc.tensor.matmul(o1[0:65, pws:], rr[:, h0 + 1, :],
                             pp[:, 512 + pws:], start=(pi == 0), stop=True)
            # normalization, decoupled from the PSUM banks: o and the
            # denominators drain to SBUF on DVE (fast bank release), 1/d via
            # the DVE fast-reciprocal custom op (no ACT involvement), then
            # broadcast + multiply run entirely on the idle gpsimd engine.
            at = att_tiles[(hp, j)]
            dd0 = nrm_pool.tile([1, 512], F32, name="dd0", tag="nrm")
            dd1 = nrm_pool.tile([1, 512], F32, name="dd1", tag="nrm")
            nc.vector.tensor_copy(dd0, o0[64:65, :])
            nc.vector.tensor_copy(dd1, o1[64:65, :])
            r0 = nrm_pool.tile([1, 512], F32, name="r0", tag="nrm")
            r1 = nrm_pool.tile([1, 512], F32, name="r1", tag="nrm")
            nc.vector.reciprocal_approx_fast(r0, dd0)
            nc.vector.reciprocal_approx_fast(r1, dd1)
            rb0 = rb_pool.tile([64, 512], F32, name="rb0", tag="rb")
            rb1 = rb_pool.tile([64, 512], F32, name="rb1", tag="rb")
            nc.gpsimd.partition_broadcast(rb0, r0)
            nc.gpsimd.partition_broadcast(rb1, r1)
            nc.vector.tensor_tensor(at[0:64, :], o0[0:64, :], rb0, op=OP.mult)
            nc.vector.tensor_tensor(at[64:128, :], o1[0:64, :], rb1, op=OP.mult)

        # j-blocks interleaved per head pair so the qkT producer stream
        # spreads over the whole attention phase (one f-pair per hp); the
        # j=0 projection pumps into the last pair's j=1 attention.
        for hp in range(8):
            ensure_qk(hp), ensure_qk(8 + hp)
            attention(0, hp, PUMP_J0)
            if hp == 7:
                prod.append(("proj", 0))
            attention(1, hp, PUMP_J1)
        pump(10 ** 6)  # flush leftovers
        for u in proj_units(1):
            u()


def _pin_act_table(arch):
    """Force every ACT func we use into one table so walrus never emits
    mid-kernel ACT_TABLE_LOADs (each is ~1.3us on the ScalarE stream)."""
    import concourse.hw_specs as hw_specs
    tabs = hw_specs.get_activation_tables(arch)
    keep = "natural_log_exp_and_others"
    if keep not in tabs:
        return
    need = tabs[keep] & {AF.Exp, AF.Ln, AF.Copy, AF.Identity}
    for name, fns in tabs.items():
        if name != keep:
            fns -= need


def _get_nc():
    if "nc" in _CACHE:
        return _CACHE["nc"]
    nc = bacc.Bacc("TRN2", target_bir_lowering=False, debug=False,
                   num_devices=N_CORES)
    _pin_act_table(nc.m.arch)
    _patch_ldw_opt()
    aps = {
        "x": nc.dram_tensor("x", [T, D], F32, kind="ExternalInput").ap(),
        "w_qkv": nc.dram_tensor("w_qkv", [D, 3 * D], BF16, kind="ExternalInput").ap(),
        "b_qkv": nc.dram_tensor("b_qkv", [3 * D], F32, kind="ExternalInput").ap(),
        "w_proj": nc.dram_tensor("w_proj", [D, D], BF16, kind="ExternalInput").ap(),
        "ident": nc.dram_tensor("ident", [P, P], F32, kind="ExternalInput").ap(),
        "tri": nc.dram_tensor("tri", [P, P], BF16, kind="ExternalInput").ap(),
        "bqv_bc": nc.dram_tensor("bqv_bc", [1, D], F32, kind="ExternalInput").ap(),
        "bp_bc": nc.dram_tensor("bp_bc", [1, D], F32, kind="ExternalInput").ap(),
        "out": nc.dram_tensor("out", [T, D], F32, kind="ExternalOutput").ap(),
    }
    _build_tile_kernel(nc, aps)
    nc.compile()
    _CACHE["nc"] = nc
    return nc


def _host_consts():
    import ml_dtypes
    ident = np.eye(P, dtype=np.float32)
    r = np.arange(P)
    tri = (r[:, None] <= r[None, :]).astype(ml_dtypes.bfloat16)
    return ident, tri


def kernel(x, w_qkv, b_qkv, w_proj, b_proj):
    x = np.ascontiguousarray(np.asarray(x, dtype=np.float32))
    w_qkv = np.ascontiguousarray(np.asarray(w_qkv, dtype=np.float32))
    b_qkv = np.ascontiguousarray(np.asarray(b_qkv, dtype=np.float32))
    w_proj = np.ascontiguousarray(np.asarray(w_proj, dtype=np.float32))
    b_proj = np.ascontiguousarray(np.asarray(b_proj, dtype=np.float32))

    nc = _get_nc()
    import ml_dtypes
    bf = ml_dtypes.bfloat16
    ident, tri = _host_consts()
    wq_bf = w_qkv.astype(bf)
    wp_bf = w_proj.astype(bf)
    bqv_bc = np.ascontiguousarray(b_qkv[2048:3072].reshape(1, D))
    bp_bc = np.ascontiguousarray(b_proj.reshape(1, D))
    in_maps = [
        {
            "x": x[b],
            "w_qkv": wq_bf,
            "b_qkv": b_qkv,
            "w_proj": wp_bf,
            "ident": ident,
            "tri": tri,
            "bqv_bc": bqv_bc,
            "bp_bc": bp_bc,
        }
        for b in range(N_CORES)
    ]
    res = bass_utils.run_bass_kernel_spmd(
        nc, in_maps, core_ids=list(range(N_CORES)), trace=TRACE
    )
    LAST_RESULT["res"] = res
    return np.stack([res.results[c]["out"] for c in range(N_CORES)]).astype(
        np.float32
    )
